# revision 75
# baseline (speedup 1.0000x reference)
"""Causal multi-head attention (B=4, H=16, S=2048, D=64) on 8 TRN2 NeuronCores.

Sharding: 64 (batch, head) pairs, 8 per core, processed as 4 "duos" (X, Y).
q/k are host-pre-transposed to d-major and duo-stacked: X's 64 d-rows on
SBUF partitions 0-63, Y's on 64-127.

Per-duo algorithm (flash-attention, transposed-score layout), per q-tile
("unit", 512 q cols), over k-tiles t in the causal lower triangle:

S^T stage - ROW-TILED matmul pairs: X's S^T at tile_position (0,0) using PE
rows 0-63, Y's at (64,0) using rows 64-127. The two contraction-64 matmuls
run concurrently in the PE array, writing one [128, 2, 512] two-bank PSUM
super-tile; full-array activity also keeps the HAM clock monitor at K=8/8
(2.4 GHz) - half-array streams measurably never leave the 1.2 GHz cold
clock (the v1 kernel's core defect: every matmul ran at (219+N)/1.2 ns).

exp - one wide instruction per k-tile covering both streams ((N+352)/1.2 ns
on ScalarE: batching amortizes the 352-cycle overhead). ScalarE alone
(~46us/duo) would cap the kernel below the PE's ~30us/duo, so k-tiles
alternate between ScalarE exact exp and a VectorE Schraudolph bit-trick
exp: int16(round(x*128*log2e*scale + (16256 - 128*0.043677))) bitcast to
bf16 ~= exp(x*scale) within +-3 percent (measured end-to-end rel-err 0.008
at a 0.02 gate; all-ScalarE would be 0.003). EXP_PATTERN must alternate
strictly - two adjacent "s" slots overload ScalarE within one pipeline
group and cost ~4-30us. Causal masks on the diagonal tiles: a DVE
tensor_tensor multiply with a precomputed 0/1 triangle when the tile's exp
ran on the DVE (same engine, no cross-engine hop), else a single GPSIMD
affine_select over both streams' first 128 columns. Diagonal k-tiles are
SPREAD evenly through each unit (one per group): front-loading them
serializes their exp->mask chains past what the PE can cover, and they
must stay off the end-of-unit flush (PV summation order is commutative).

PV stage - acc[65, 512] += V'[128, 65]^T @ P[128, w] per stream; V' carries
a ones column so acc row 64 accumulates the softmax denominator for free.
PV-pair emission lags the S-pairs by LAG=5 k-tiles and carries ACROSS unit
boundaries (the [4xS][4xPV] batching is preserved - strict alternation
breaks MM chaining), so the end-of-unit PV flush never starves the exp
engines; 3 ps super-tile buffers cover the lag and the tail transpose
scratch is carved from a ps slot's banks to stay within 8 PSUM banks.
Tails are emitted only at group boundaries once their unit's PVs are all
emitted - a tail matmul landing mid-PV-run breaks chaining, and a tail
copy emitted too early head-of-line-blocks its engine's exp queue.

Unit tails (both streams fused per unit): evict accX/accY -> one SBUF bf16
tile (VectorE x2), transpose via 8 identity matmuls into a 2-bank ps slot
[128, 2, 4, 65], ONE strided reciprocal of the 8 denominator columns, ONE
broadcast normalize mul, ONE bf16 DMA whose DRAM rows are
(unit, duo, p, stream, block) so each partition writes a 1KB contiguous
run (assemble() undoes the permutation on host; bf16 out costs ~1e-4 of
rel err and halves the final drain).

DMA layout notes (v2): v is host-pre-shuffled to [duo*kp, (s, t, d+1)]
with the ones column baked in, so each duo's V loads as ONE fully
contiguous 4KB-per-partition transfer (the old per-head gather moved
512KB/duo in 128B packets and its descriptor generation occupied the sync
queue for 5.7us; total run packets dropped 34863 -> 4511, which also
removed the duo-boundary HAM half-clock dips). The first duo's j=0
columns are split across the three DMA-capable queues (sync/scalar q
halves, gpsimd first k-tile, 32KB) so the PE starts at ~10.4us - bounded
below by the ~7us framework preamble (two barrier rounds + per-engine
tensor_loads; fixed) plus ~2.5us of DMA-engine wake+transfer latency.

Measured traps (do not re-try): issuing tail transposes as DMA_TRANSPOSE
XBAR ops costs 1.2us of sync-engine issue each and head-of-line-blocks
later duo loads (+7us); moving all diagonal masks to gpsimd adds a
cross-engine exp->mask hop (+1.5us); splitting the acc eviction across
scalar+vector loads the scalar exp queue (+3us); LAG=6 (+7us, delays the
end-of-unit flush past the acc recycle); GR=3 so S-runs fit the 3 ps
slots (+8us: more S<->PV mode switches per tile - the ~130ns switch cost
is per PE mode transition, and PP/TT transitions are nearly free);
halving the tail evictions to bound DVE head-of-line blocking (+8us:
PSUM-read fixed overhead per DVE op dominates); fusing accX/accY into one
2-bank acc tile to halve eviction copies (+14us cool-state: tile-granular
dependency tracking serializes the two PV chains through the shared
tile); tail-priority delay 120 (+3us). S<->PV transitions cost ~120-140ns each (~24us/run at ~52/duo vs
~20 ideal) - the scheduler fragments the [4S][8P] blocks whenever an exp
is late, but every attempt to force consolidation lost more than it won.

Row-tiled PV (the would-be fix for the S<->PV switch tax) is measurably
dead: splitting each PV into two K=64 quadrant passes needs either (a)
two acc banks per stream summed at eviction - but a DVE op may read only
ONE input from PSUM (NCC_IBVF027), and an extra copy costs +21us of DVE -
or (b) cross-stream half pairing (X-low with Y-high, then mirrored) into
the existing two banks via open accumulation chains - which HANGS the
device: quadrant-mode (tile_position) matmuls evidently cannot leave a
PSUM accumulation group open across instructions (every working row-tiled
matmul here is start=True stop=True; the full-array PV chains are what
carry start/stop).

Thermal protocol: sustained benching drives the chip into a throttle
state where ScalarE/DVE run ~20% slow while the PE loses only ~3%
(ACTIVATE avg 1041 -> 1247ns) - wall time inflates ~150 -> ~180us with
IDENTICAL code. Cool-down takes minutes of idle. Never conclude from
absolute numbers across a session; compare only within a tight window,
checking ACTIVATE-avg as the clock-state canary.
"""

import math

import numpy as np
import ml_dtypes

import concourse.bass as bass
import concourse.bacc as bacc
import concourse.tile as tile
import concourse.mybir as mybir
from concourse import bass_utils
from concourse.masks import make_identity

B, H, S, D = 4, 16, 2048, 64
N_CORES = 8
PAIRS = (B * H) // N_CORES  # 8 heads per core
DUOS = PAIRS // 2           # 4 lockstep duos per core
QT = 512                    # q-tile width
KT = 128                    # k-tile rows
NQT = S // QT               # 4 q-tiles per head
GR = 4                      # k-tiles per pipeline group
SCALE = 1.0 / math.sqrt(D)
A_SCH = (128.0 / math.log(2.0)) * SCALE       # Schraudolph slope (scale folded)
B_SCH = 16256.0 - 128.0 * 0.043677            # Schraudolph offset (bf16 bias)
EXP_PATTERN = ("s", "v", "s", "v", "s")  # 3/5 ScalarE exact (tuned)
BF16 = ml_dtypes.bfloat16

_COMPILED = {}


def build_nc():
    nc = bacc.Bacc(
        "TRN2",
        target_bir_lowering=False,
        debug=False,
        enable_asserts=False,
        num_devices=N_CORES,
    )
    f32 = mybir.dt.float32
    bf16 = mybir.dt.bfloat16
    i16 = mybir.dt.int16

    qt_d = nc.dram_tensor("qt", [DUOS * 2 * D, S], bf16, kind="ExternalInput").ap()
    kt_d = nc.dram_tensor("kt", [DUOS * 2 * D, S], bf16, kind="ExternalInput").ap()
    # v pre-shuffled on host to [kp, (s, t, d+1)] per duo, ones column
    # pre-filled: the whole duo streams as one contiguous 4KB/partition DMA
    v_d = nc.dram_tensor(
        "v", [DUOS * KT, 2 * (S // KT) * (D + 1)], bf16,
        kind="ExternalInput").ap()
    out_d = nc.dram_tensor("out", [PAIRS * S, D], bf16,
                           kind="ExternalOutput").ap()

    with tile.TileContext(nc) as tc:
        with (
            tc.tile_pool(name="consts", bufs=1) as consts,
            tc.tile_pool(name="qk", bufs=2) as qk_pool,
            tc.tile_pool(name="vp", bufs=2) as v_pool,
            tc.tile_pool(name="pp", bufs=12) as p_pool,
            tc.tile_pool(name="op", bufs=4) as o_pool,
            tc.tile_pool(name="fp", bufs=4) as f_pool,
            tc.tile_pool(name="rp", bufs=4) as r_pool,
            tc.tile_pool(name="big", bufs=3, space="PSUM") as big_pool,
            tc.tile_pool(name="acc", bufs=2, space="PSUM") as acc_pool,
        ):
            st = {"exp": 0, "tail": 0}

            def load_duo(dd):
                qsb = qk_pool.tile([2 * D, S], bf16, tag="qsb", name=f"q{dd}")
                ksb = qk_pool.tile([2 * D, S], bf16, tag="ksb", name=f"k{dd}")
                r0, r1 = dd * 128, (dd + 1) * 128
                if dd == 0:
                    # first duo: land the j=0 columns first and SPREAD them
                    # over the three DMA-capable queues so the PE starts
                    # ASAP; the first k tile is its own 32KB transfer so the
                    # first S-pair's wait is minimal
                    nc.sync.dma_start(out=qsb[:, 0:QT // 2],
                                      in_=qt_d[r0:r1, 0:QT // 2])
                    nc.scalar.dma_start(out=qsb[:, QT // 2:QT],
                                        in_=qt_d[r0:r1, QT // 2:QT])
                    nc.gpsimd.dma_start(out=ksb[:, 0:KT],
                                        in_=kt_d[r0:r1, 0:KT])
                    nc.scalar.dma_start(out=ksb[:, KT:QT],
                                        in_=kt_d[r0:r1, KT:QT])
                    nc.sync.dma_start(out=qsb[:, QT:], in_=qt_d[r0:r1, QT:])
                    nc.scalar.dma_start(out=ksb[:, QT:], in_=kt_d[r0:r1, QT:])
                else:
                    nc.sync.dma_start(out=qsb, in_=qt_d[r0:r1, :])
                    nc.sync.dma_start(out=ksb, in_=kt_d[r0:r1, :])
                vt = v_pool.tile([KT, 2, S // KT, D + 1], bf16, tag="v",
                                 name=f"v{dd}")
                nc.sync.dma_start(
                    out=vt,
                    in_=v_d[dd * KT:(dd + 1) * KT, :].rearrange(
                        "kp (s t c) -> kp s t c", s=2, t=S // KT),
                )
                vs = [vt[:, 0, :, :], vt[:, 1, :, :]]
                return qsb, ksb, vs

            def emit_s(sb, j, t, halves=1):
                qsb, ksb, _ = sb
                off = max(0, KT * t - QT * j)
                w = QT - off
                q0 = QT * j + off
                ps = big_pool.tile([KT, 2, QT], f32, tag="ps", name="ps")
                # halves>1 only for the very first pair: its first half needs
                # just q[0:256]+k[0:128], so the PE starts one DMA earlier
                hw_ = w // halves
                for h_ in range(halves):
                    nc.tensor.matmul(
                        ps[:, 0, h_ * hw_:(h_ + 1) * hw_],
                        lhsT=ksb[0:D, KT * t:KT * (t + 1)],
                        rhs=qsb[0:D, q0 + h_ * hw_:q0 + (h_ + 1) * hw_],
                        start=True, stop=True, tile_position=(0, 0),
                    )
                    nc.tensor.matmul(
                        ps[:, 1, h_ * hw_:(h_ + 1) * hw_],
                        lhsT=ksb[D:2 * D, KT * t:KT * (t + 1)],
                        rhs=qsb[D:2 * D, q0 + h_ * hw_:q0 + (h_ + 1) * hw_],
                        start=True, stop=True, tile_position=(64, 0),
                    )
                p3 = p_pool.tile([KT, 2, QT], bf16, tag="p3", name="p3")
                eng = EXP_PATTERN[st["exp"] % len(EXP_PATTERN)]
                st["exp"] += 1
                if eng == "s":
                    nc.scalar.activation(
                        out=p3[:, :, 0:w], in_=ps[:, :, 0:w],
                        func=mybir.ActivationFunctionType.Exp, scale=SCALE,
                    )
                else:
                    nc.vector.tensor_scalar(
                        out=p3[:, :, 0:w].bitcast(i16), in0=ps[:, :, 0:w],
                        scalar1=A_SCH, scalar2=B_SCH,
                        op0=mybir.AluOpType.mult, op1=mybir.AluOpType.add,
                    )
                if t >= (QT // KT) * j:  # diagonal tile: zero q_rel < k_rel
                    if eng == "v":
                        # same engine as the exp: no cross-engine hop, and
                        # it halves the gpsimd mask serialization
                        nc.vector.tensor_tensor(
                            out=p3[:, :, 0:KT], in0=p3[:, :, 0:KT],
                            in1=trimask, op=mybir.AluOpType.mult,
                        )
                    else:
                        nc.gpsimd.affine_select(
                            out=p3[:, :, 0:KT], in_=p3[:, :, 0:KT],
                            compare_op=mybir.AluOpType.is_ge,
                            fill=0.0, base=0,
                            pattern=[[0, 2], [1, KT]], channel_multiplier=-1,
                        )
                return p3, off, w

            def emit_pv(sb, accs, t, p3off, first, last):
                p3, off, w = p3off
                for s_ in range(2):
                    nc.tensor.matmul(
                        accs[s_][:, off:QT],
                        lhsT=sb[2][s_][:, t, :],
                        rhs=p3[:, s_, 0:w],
                        start=first, stop=last,
                    )

            TAIL_DELAY = 0  # scheduler priority offset: tails yield to exps

            def emit_tail(dd, j, accx, accy, split=False):
                tc.cur_priority += TAIL_DELAY
                # both streams' tails fused: one transpose scratch (both
                # banks of a ps slot), one reciprocal, one normalize mul,
                # one bf16 output DMA covering heads 2dd and 2dd+1
                osb = o_pool.tile([D + 1, 2, QT], bf16, tag="osb", name="osb")
                nc.vector.tensor_copy(osb[:, 0, :], accx)
                nc.vector.tensor_copy(osb[:, 1, :], accy)
                st["tail"] += 1
                pst = big_pool.tile([KT, 2, QT], f32, tag="ps", name="tpslot")
                tp = pst[:, :, 0:(QT // KT) * (D + 1)].rearrange(
                    "p s (b c) -> p s b c", b=QT // KT)
                for s_ in range(2):
                    for b_ in range(QT // KT):
                        nc.tensor.matmul(
                            tp[:, s_, b_, :],
                            lhsT=osb[:, s_, KT * b_:KT * (b_ + 1)],
                            rhs=ident, start=True, stop=True,
                        )
                rinv = r_pool.tile([KT, 2, QT // KT], f32, tag="ri",
                                   name="rinv")
                nc.vector.reciprocal(rinv, tp[:, :, :, D])
                fsb = f_pool.tile([KT, 2, QT // KT, D], bf16, tag="f",
                                  name="fsb")
                # one mul for all 8 blocks: rinv broadcast along d (stride 0)
                nc.vector.tensor_tensor(
                    out=fsb, in0=tp[:, :, :, 0:D],
                    in1=rinv.broadcast_to([KT, 2, QT // KT, D]),
                    op=mybir.AluOpType.mult,
                )
                # device-side out rows are (unit, duo, p, stream, block):
                # each partition writes one 1KB contiguous DRAM run;
                # assemble() reorders on host
                r = (j * DUOS + dd) * 2 * QT
                oap = out_d[r:r + 2 * QT].rearrange(
                    "(p s b) d -> p s b d", p=KT, s=2)
                if split:
                    # end-of-run tails: two queues drain in parallel
                    nc.sync.dma_start(out=oap[:, 0], in_=fsb[:, 0])
                    nc.scalar.dma_start(out=oap[:, 1], in_=fsb[:, 1])
                else:
                    nc.sync.dma_start(out=oap, in_=fsb)
                tc.cur_priority -= TAIL_DELAY

            sbs = load_duo(0)
            # consts AFTER the first duo's DMA issue: the gpsimd/scalar
            # queue work here must not delay the startup transfers.
            # 0/1 causal triangle for DVE-side diagonal masking
            trimask = consts.tile([KT, 2, KT], bf16)
            nc.gpsimd.memset(trimask, 1.0)
            nc.gpsimd.affine_select(
                out=trimask, in_=trimask,
                compare_op=mybir.AluOpType.is_ge, fill=0.0, base=0,
                pattern=[[0, 2], [1, KT]], channel_multiplier=-1,
            )
            ident = consts.tile([D + 1, D + 1], bf16)
            make_identity(nc, ident)
            sbs_next = None
            LAG = GR + 1
            pend = []           # (sb, accs, nkt, t, p3off, uid)
            npop = {}           # uid -> PVs emitted so far
            pending_tails = []  # (uid, dd, j, accx, accy)

            def pop_pv():
                psb, paccs, pnkt, tt, p3off, uid = pend.pop(0)
                k = npop.get(uid, 0)
                emit_pv(psb, paccs, tt, p3off,
                        first=(k == 0), last=(k == pnkt - 1))
                npop[uid] = k + 1

            def emit_ready_tails():
                # a unit's tails may go once all its PVs are emitted; placed
                # at group boundaries so the transpose matmuls never land
                # mid-PV-run (which breaks MM chaining)
                while pending_tails and npop.get(pending_tails[0][0], 0) == \
                        4 * (pending_tails[0][2] + 1):
                    emit_tail(*pending_tails.pop(0)[1:])

            for dd in range(DUOS):
                sb = sbs
                for j in range(NQT):
                    uid = dd * NQT + j
                    nkt = (QT // KT) * (j + 1)
                    accx = acc_pool.tile([D + 1, QT], f32, tag="acc",
                                         name="accx")
                    accy = acc_pool.tile([D + 1, QT], f32, tag="acc",
                                         name="accy")
                    accs = (accx, accy)
                    # spread diagonal k-tiles evenly (one per group):
                    # front-loading all 4 serializes their exp->mask chains
                    # past what the PE can cover, and they must stay off the
                    # end-of-unit flush
                    fi = iter(range(4 * j))
                    t_order = []
                    for dt in range(4 * j, nkt):
                        t_order.append(dt)
                        for _ in range(j):
                            t_order.append(next(fi))
                    for g0 in range(0, nkt, GR):
                        for t in t_order[g0:g0 + GR]:
                            pend.append((sb, accs, nkt, t,
                                         emit_s(sb, j, t), uid))
                        if g0 == 0 and j == 1 and dd + 1 < DUOS:
                            sbs_next = load_duo(dd + 1)
                        while len(pend) > LAG:
                            pop_pv()
                        emit_ready_tails()
                    pending_tails.append((uid, dd, j, accx, accy))
                sbs = sbs_next
            while pend:
                pop_pv()
            for args in pending_tails:
                emit_tail(*args[1:])

    nc.compile()
    return nc


def _get_nc():
    if "nc" not in _COMPILED:
        _COMPILED["nc"] = build_nc()
    return _COMPILED["nc"]


def make_in_maps(q, k, v):
    q = np.asarray(q, dtype=np.float32).reshape(B * H, S, D)
    k = np.asarray(k, dtype=np.float32).reshape(B * H, S, D)
    v = np.asarray(v, dtype=np.float32).reshape(B * H, S, D)
    in_maps = []
    for c in range(N_CORES):
        sl = slice(c * PAIRS, (c + 1) * PAIRS)
        # duo-stacked d-major [DUOS*128, S]: duo dd rows 0-63 = head 2dd,
        # rows 64-127 = head 2dd+1
        qt = np.ascontiguousarray(q[sl].transpose(0, 2, 1)).reshape(
            DUOS * 2 * D, S)
        kt = np.ascontiguousarray(k[sl].transpose(0, 2, 1)).reshape(
            DUOS * 2 * D, S)
        # v shuffled to [duo*kp, (s, t, d+1)] with the ones column baked in
        vc = v[sl].reshape(DUOS, 2, S // KT, KT, D).transpose(0, 3, 1, 2, 4)
        vp = np.ones((DUOS, KT, 2, S // KT, D + 1), dtype=np.float32)
        vp[..., 0:D] = vc
        in_maps.append({
            "qt": qt.astype(BF16),
            "kt": kt.astype(BF16),
            "v": vp.reshape(DUOS * KT, 2 * (S // KT) * (D + 1)).astype(BF16),
        })
    return in_maps


def assemble(results):
    out = np.empty((B * H, S, D), dtype=np.float32)
    for c in range(N_CORES):
        # device rows are (unit, duo, p, stream, block); see emit_tail
        arr = results[c]["out"].reshape(NQT, DUOS, KT, 2, QT // KT, D)
        out[c * PAIRS:(c + 1) * PAIRS] = arr.transpose(
            1, 3, 0, 4, 2, 5).reshape(PAIRS, S, D)
    return np.ascontiguousarray(
        out.reshape(B, H, S, D).transpose(0, 2, 1, 3).reshape(B, S, H * D))


def kernel(q, k, v):
    nc = _get_nc()
    res = bass_utils.run_bass_kernel_spmd(
        nc, make_in_maps(q, k, v), core_ids=list(range(N_CORES)))
    return assemble(res.results)

